# revision 6
# baseline (speedup 1.0000x reference)
"""Trainium2 Bass kernel for a dense GAT layer.

Reference semantics (jax):
    h     = x @ W.T                          # (N, F_OUT)
    s_src = h @ a[:F_OUT]                    # (N,)   per-dest-row bias
    s_dst = h @ a[F_OUT:]                    # (N,)   per-source-col bias
    e     = leaky_relu(s_src[:,None] + s_dst[None,:], 0.2)
    alpha = softmax(e, axis=1)               # (N, N)
    out   = alpha @ h                        # (N, F_OUT)
    return (out, alpha)

Sharding: rows of the N x N attention (query/destination nodes) are split
across the 8 NeuronCores. Each core receives the full x (replicated) plus its
own 1024-row slice, computes the full h redundantly (cheap), and produces its
1024-row block of alpha and out. Host concatenates the blocks.

Per-core device algorithm (all f32):
  setup:
    xT   = transpose(x) via PE                          [F_IN, N]
    h    = x @ W.T  (lhsT=xT chunks, rhs=W^T chunks)    [j-on-partitions]
    w0/w1 = W^T @ a halves (tiny)
    t    = x @ w1 as a [1, N] row, PE-broadcast to t_bc [128, N]
    s    = xb @ w0 -> per-partition bias per i-tile     [128, NIT]
  main loop over the core's 8 row-tiles (128 rows each):
    u     = Prelu(t_bc + s_i)         (ScalarE, slope 0.2, bias per-partition)
    p     = Exp(u), accum_out -> D_i  (ScalarE; row sums for softmax denom)
    alpha = p * (1/D_i)               (VectorE tensor_scalar) -> DMA out
    pT    = PE transpose of p chunks -> PSUM -> SBUF
    numT += h_j^T-free matmuls: numT[fo,i] = sum_j h[j,fo]^T p^T[j,i] (PSUM acc)
    out   = transpose(numT) * (1/D_i) -> DMA out
"""

import os
import sys

for _p in ("/opt/trn_rl_repo",):
    if os.path.isdir(_p) and _p not in sys.path:
        sys.path.insert(0, _p)

from contextlib import ExitStack

import numpy as np

import concourse.bass as bass
import concourse.tile as tile
from concourse import mybir
from concourse.masks import make_identity

AF = mybir.ActivationFunctionType
F32 = mybir.dt.float32

N_TOT = 8192
F_IN = 256
F_OUT = 128
N_CORES = 8
BLK = N_TOT // N_CORES
SLOPE = 0.2


def build_nc(n_tot=N_TOT, blk=BLK, f_in=F_IN, f_out=F_OUT, slope=SLOPE,
             legalize=True):
    """Build the single-core SPMD Bass program (same program on all cores;
    per-core behavior differs only through the `xb` input slice)."""
    P = 128
    assert f_out == P and f_in % P == 0
    FI_C = f_in // P            # contraction chunks over F_IN (2)
    NJT = n_tot // P            # source-node (column) tiles (64)
    NIT = blk // P              # this core's dest-row tiles (8)
    JH = n_tot // 2             # j extent per half in the main loop
    GRP = 4                     # 128-wide j-chunks per PSUM bank group
    NGH = JH // (GRP * P)       # transpose groups per half

    nc = bass.Bass()

    x_d = nc.dram_tensor("x", [n_tot, f_in], F32, kind="ExternalInput")
    xb_d = nc.dram_tensor("xb", [blk, f_in], F32, kind="ExternalInput")
    W_d = nc.dram_tensor("W", [f_out, f_in], F32, kind="ExternalInput")
    a0_d = nc.dram_tensor("a0", [f_out, 1], F32, kind="ExternalInput")
    a1_d = nc.dram_tensor("a1", [f_out, 1], F32, kind="ExternalInput")
    alpha_d = nc.dram_tensor("alpha_blk", [blk, n_tot], F32, kind="ExternalOutput")
    out_d = nc.dram_tensor("out_blk", [blk, f_out], F32, kind="ExternalOutput")

    with tile.TileContext(nc) as tc, ExitStack() as ctx:
        const = ctx.enter_context(tc.tile_pool(name="const", bufs=1))

        ident = const.tile([P, P], F32)
        make_identity(nc, ident)
        W_s = const.tile([P, f_in], F32)
        nc.sync.dma_start(out=W_s, in_=W_d[:, :])
        a0_s = const.tile([P, 1], F32)
        nc.sync.dma_start(out=a0_s, in_=a0_d[:, :])
        a1_s = const.tile([P, 1], F32)
        nc.sync.dma_start(out=a1_s, in_=a1_d[:, :])
        ones_s = const.tile([1, P], F32)
        nc.vector.memset(ones_s, 1.0)

        WT_s = const.tile([P, f_in], F32)       # W^T, chunk c at [:, c*P:(c+1)*P]
        w0_s = const.tile([P, FI_C], F32)       # W^T @ a0, chunk c at [:, c]
        w1_s = const.tile([P, FI_C], F32)
        h_s = const.tile([P, NJT * P], F32)     # h natural: [j%P, (j//P)*P + fo]
        s_s = const.tile([P, NIT], F32)         # per-row bias s_i for our block
        t_bc = const.tile([P, n_tot], F32)      # t broadcast across partitions

        # ---------------- setup ----------------
        with tc.tile_pool(name="su_ps", bufs=2, space="PSUM") as sps:
            # W^T via PE transpose
            wt_ps = sps.tile([P, f_in], F32, tag="wt")
            for c in range(FI_C):
                nc.tensor.transpose(
                    wt_ps[:, c * P:(c + 1) * P], W_s[:, c * P:(c + 1) * P], ident
                )
            nc.vector.tensor_copy(out=WT_s, in_=wt_ps)

            # w0 = W^T a0, w1 = W^T a1 (chunked over fi)
            w01_ps = sps.tile([P, 2 * FI_C], F32, tag="w01")
            for c in range(FI_C):
                nc.tensor.matmul(
                    w01_ps[:, c:c + 1], lhsT=W_s[:, c * P:(c + 1) * P], rhs=a0_s,
                    start=True, stop=True, skip_group_check=True,
                )
                nc.tensor.matmul(
                    w01_ps[:, FI_C + c:FI_C + c + 1],
                    lhsT=W_s[:, c * P:(c + 1) * P], rhs=a1_s,
                    start=True, stop=True, skip_group_check=True,
                )
            nc.vector.tensor_copy(out=w0_s, in_=w01_ps[:, 0:FI_C])
            nc.vector.tensor_copy(out=w1_s, in_=w01_ps[:, FI_C:2 * FI_C])

        with tc.tile_pool(name="xT_sb", bufs=1) as xtp:
            # xT chunks: xT[c] = x[:, c*P:(c+1)*P].T, shape [P(fi), n_tot(j)]
            xT = [xtp.tile([P, n_tot], F32, tag=f"xT{c}", name=f"xT{c}")
                  for c in range(FI_C)]

            with tc.tile_pool(name="xl_sb", bufs=3) as xlp, \
                 tc.tile_pool(name="xt_ps", bufs=2, space="PSUM") as xtps, \
                 tc.tile_pool(name="h_ps", bufs=2, space="PSUM") as hps:
                NG = NJT // GRP
                for g in range(NG):
                    pts = [xtps.tile([P, GRP * P], F32, tag=f"xt{c}", name=f"xt{c}")
                           for c in range(FI_C)]
                    for k in range(GRP):
                        jt = g * GRP + k
                        x_t = xlp.tile([P, f_in], F32, tag="x")
                        nc.sync.dma_start(out=x_t, in_=x_d[jt * P:(jt + 1) * P, :])
                        for c in range(FI_C):
                            nc.tensor.transpose(
                                pts[c][:, k * P:(k + 1) * P],
                                x_t[:, c * P:(c + 1) * P], ident,
                            )
                    # split PSUM->SBUF copies between DVE and ACT
                    nc.vector.tensor_copy(
                        out=xT[0][:, g * GRP * P:(g + 1) * GRP * P], in_=pts[0])
                    nc.scalar.copy(
                        out=xT[1][:, g * GRP * P:(g + 1) * GRP * P], in_=pts[1])

                # h natural: h[j, fo] accumulated over fi chunks
                for g in range(NG):
                    h_ps = hps.tile([P, GRP * P], F32, tag="h")
                    for k in range(GRP):
                        jt = g * GRP + k
                        for c in range(FI_C):
                            nc.tensor.matmul(
                                h_ps[:, k * P:(k + 1) * P],
                                lhsT=xT[c][:, jt * P:(jt + 1) * P],
                                rhs=WT_s[:, c * P:(c + 1) * P],
                                start=(c == 0), stop=(c == FI_C - 1),
                                skip_group_check=True,
                            )
                    nc.vector.tensor_copy(
                        out=h_s[:, g * GRP * P:(g + 1) * GRP * P], in_=h_ps)

            # t row vector [1, n_tot] then PE-broadcast into t_bc [P, n_tot]
            with tc.tile_pool(name="t_ps", bufs=2, space="PSUM") as tps, \
                 tc.tile_pool(name="t_sb", bufs=1) as tsb, \
                 tc.tile_pool(name="bc_ps", bufs=2, space="PSUM") as bcps:
                t_free = tsb.tile([1, n_tot], F32)
                NT5 = n_tot // 512
                for n in range(NT5):
                    t_ps = tps.tile([1, 512], F32, tag="t")
                    for c in range(FI_C):
                        nc.tensor.matmul(
                            t_ps, lhsT=w1_s[:, c:c + 1],
                            rhs=xT[c][:, n * 512:(n + 1) * 512],
                            start=(c == 0), stop=(c == FI_C - 1),
                            skip_group_check=True,
                        )
                    nc.scalar.copy(out=t_free[:, n * 512:(n + 1) * 512], in_=t_ps)
                for n in range(NT5):
                    bc_ps = bcps.tile([P, 512], F32, tag="bc")
                    nc.tensor.matmul(
                        bc_ps, lhsT=ones_s, rhs=t_free[:, n * 512:(n + 1) * 512],
                        start=True, stop=True, skip_group_check=True,
                    )
                    nc.vector.tensor_copy(
                        out=t_bc[:, n * 512:(n + 1) * 512], in_=bc_ps)

        # s_i for our row block: s = xb @ w0, via transposed xb chunks
        with tc.tile_pool(name="s_ps", bufs=2, space="PSUM") as ssps, \
             tc.tile_pool(name="s_sb", bufs=2) as ssb:
            xbT = [ssb.tile([P, blk], F32, tag=f"xbT{c}", name=f"xbT{c}")
                   for c in range(FI_C)]
            NGB = NIT // GRP if NIT >= GRP else 1
            BG = min(GRP, NIT)
            for g in range(NGB):
                pts = [ssps.tile([P, BG * P], F32, tag=f"bt{c}", name=f"bt{c}")
                       for c in range(FI_C)]
                for k in range(BG):
                    it = g * BG + k
                    xb_t = ssb.tile([P, f_in], F32, tag="xb")
                    nc.sync.dma_start(out=xb_t, in_=xb_d[it * P:(it + 1) * P, :])
                    for c in range(FI_C):
                        nc.tensor.transpose(
                            pts[c][:, k * P:(k + 1) * P],
                            xb_t[:, c * P:(c + 1) * P], ident,
                        )
                for c in range(FI_C):
                    nc.vector.tensor_copy(
                        out=xbT[c][:, g * BG * P:(g + 1) * BG * P], in_=pts[c])
            s_ps = ssps.tile([P, NIT], F32, tag="s")
            for it in range(NIT):
                for c in range(FI_C):
                    nc.tensor.matmul(
                        s_ps[:, it:it + 1],
                        lhsT=xbT[c][:, it * P:(it + 1) * P],
                        rhs=w0_s[:, c:c + 1],
                        start=(c == 0), stop=(c == FI_C - 1),
                        skip_group_check=True,
                    )
            nc.vector.tensor_copy(out=s_s, in_=s_ps)

        # ---------------- main loop ----------------
        upool = ctx.enter_context(tc.tile_pool(name="u", bufs=1))
        ppool = ctx.enter_context(tc.tile_pool(name="p", bufs=3))
        apool = ctx.enter_context(tc.tile_pool(name="al", bufs=2))
        dpool = ctx.enter_context(tc.tile_pool(name="d", bufs=2))
        ptsb = ctx.enter_context(tc.tile_pool(name="pts", bufs=3))
        opool = ctx.enter_context(tc.tile_pool(name="o", bufs=2))
        ptps = ctx.enter_context(tc.tile_pool(name="pt_ps", bufs=2, space="PSUM"))
        numps = ctx.enter_context(tc.tile_pool(name="num_ps", bufs=2, space="PSUM"))
        nops = ctx.enter_context(tc.tile_pool(name="no_ps", bufs=2, space="PSUM"))

        for it in range(NIT):
            s_bias = s_s[:, it:it + 1]
            D_t = dpool.tile([P, 2], F32, tag="D")
            numT_ps = numps.tile([P, P], F32, tag="numT")
            p_halves = []
            for hf in range(2):
                u_h = upool.tile([P, JH], F32, tag="u")
                nc.scalar.activation(
                    out=u_h, in_=t_bc[:, hf * JH:(hf + 1) * JH],
                    func=AF.Prelu, bias=s_bias, scale=1.0, alpha=slope,
                )
                p_h = ppool.tile([P, JH], F32, tag="p")
                nc.scalar.activation(
                    out=p_h, in_=u_h, func=AF.Exp,
                    accum_out=D_t[:, hf:hf + 1],
                )
                p_halves.append(p_h)
                # transpose p chunks and accumulate numT[fo, i] over j
                for g in range(NGH):
                    pt_ps = ptps.tile([P, GRP * P], F32, tag="pt")
                    for k in range(GRP):
                        jo = (g * GRP + k) * P
                        nc.tensor.transpose(
                            pt_ps[:, k * P:(k + 1) * P], p_h[:, jo:jo + P], ident)
                    pt_s = ptsb.tile([P, GRP * P], F32, tag="pts")
                    nc.vector.tensor_copy(out=pt_s, in_=pt_ps)
                    for k in range(GRP):
                        jc = hf * (JH // P) + g * GRP + k
                        nc.tensor.matmul(
                            numT_ps,
                            lhsT=h_s[:, jc * P:(jc + 1) * P],
                            rhs=pt_s[:, k * P:(k + 1) * P],
                            start=(jc == 0), stop=(jc == NJT - 1),
                            skip_group_check=True,
                        )
            # softmax denominator and normalization
            rd_t = dpool.tile([P, 1], F32, tag="rd")
            nc.vector.tensor_add(out=rd_t, in0=D_t[:, 0:1], in1=D_t[:, 1:2])
            nc.vector.reciprocal(out=rd_t, in_=rd_t)
            for hf in range(2):
                al_h = apool.tile([P, JH], F32, tag="al")
                nc.vector.tensor_scalar_mul(al_h, p_halves[hf], rd_t)
                nc.sync.dma_start(
                    out=alpha_d[it * P:(it + 1) * P, hf * JH:(hf + 1) * JH],
                    in_=al_h,
                )
            # out rows: transpose numT back and scale by 1/D
            numT_s = ptsb.tile([P, P], F32, tag="numTs")
            nc.vector.tensor_copy(out=numT_s, in_=numT_ps)
            num_ps = nops.tile([P, P], F32, tag="num")
            nc.tensor.transpose(num_ps, numT_s, ident)
            o_t = opool.tile([P, f_out], F32, tag="o")
            nc.vector.tensor_scalar_mul(o_t, num_ps, rd_t)
            nc.sync.dma_start(out=out_d[it * P:(it + 1) * P, :], in_=o_t)

    if legalize:
        _legalize_matmul_waits(nc)
    return nc


def _legalize_matmul_waits(nc):
    """This walrus build accepts only one sync-wait per instruction for
    several ISA templates (S3_LW matmul, PSEUDO_DMA_DIRECT2D, ...). Hoist
    excess waits onto NoOps on the same engine inserted right before the
    offending instruction (a wait moved earlier on the same engine is always
    safe)."""
    nop_id = 0
    for fn in nc.m.functions:
        for blk in fn.blocks:
            insts = blk.instructions
            patches = []
            for i, inst in enumerate(insts):
                si = getattr(inst, "sync_info", None)
                if si is None or len(si.on_wait) <= 1:
                    continue
                patches.append((i, inst))
            for i, inst in reversed(patches):
                si = inst.sync_info
                extra, keep = si.on_wait[:-1], si.on_wait[-1:]
                inst.sync_info = mybir.SyncInfo(on_wait=keep, on_update=si.on_update)
                for w in reversed(extra):
                    nop = mybir.InstNoOp(name=f"wsplit{nop_id}")
                    nop_id += 1
                    nop.engine = inst.engine
                    nop.sync_info = mybir.SyncInfo(on_wait=[w], on_update=[])
                    insts.insert(i, nop)


_NC_CACHE = None


def _get_nc():
    global _NC_CACHE
    if _NC_CACHE is None:
        _NC_CACHE = build_nc()
    return _NC_CACHE


def make_in_maps(x, W, a, n_cores=N_CORES, blk=BLK, f_out=F_OUT):
    x = np.ascontiguousarray(np.asarray(x, dtype=np.float32))
    W = np.ascontiguousarray(np.asarray(W, dtype=np.float32))
    a = np.ascontiguousarray(np.asarray(a, dtype=np.float32))
    a0 = a[:f_out].reshape(f_out, 1).copy()
    a1 = a[f_out:].reshape(f_out, 1).copy()
    return [
        {
            "x": x,
            "xb": np.ascontiguousarray(x[c * blk:(c + 1) * blk]),
            "W": W,
            "a0": a0,
            "a1": a1,
        }
        for c in range(n_cores)
    ]


def kernel(x, W, a):
    from concourse.bass_utils import run_bass_kernel_spmd

    nc = _get_nc()
    in_maps = make_in_maps(x, W, a)
    res = run_bass_kernel_spmd(nc, in_maps, list(range(N_CORES)))
    outs = res.results
    alpha = np.concatenate([outs[c]["alpha_blk"] for c in range(N_CORES)], axis=0)
    out = np.concatenate([outs[c]["out_blk"] for c in range(N_CORES)], axis=0)
    return out, alpha


# revision 7
# speedup vs baseline: 23.5208x; 23.5208x over previous
"""Trainium2 Bass kernel for a dense GAT layer.

Reference semantics (jax):
    h     = x @ W.T                          # (N, F_OUT)
    s_src = h @ a[:F_OUT]                    # (N,)   per-dest-row bias
    s_dst = h @ a[F_OUT:]                    # (N,)   per-source-col bias
    e     = leaky_relu(s_src[:,None] + s_dst[None,:], 0.2)
    alpha = softmax(e, axis=1)               # (N, N)
    out   = alpha @ h                        # (N, F_OUT)
    return (out, alpha)

Sharding: rows of the N x N attention (query/destination nodes) are split
across the 8 NeuronCores. Each core receives the full x (replicated) plus its
own 1024-row slice, computes the full h redundantly (cheap), and produces its
1024-row block of alpha and out. Host concatenates the blocks.

Per-core device algorithm (all f32):
  setup:
    xT   = transpose(x) via PE                          [F_IN, N]
    h    = x @ W.T  (lhsT=xT chunks, rhs=W^T chunks)    [j-on-partitions]
    w0/w1 = W^T @ a halves (tiny)
    t    = x @ w1 as a [1, N] row, PE-broadcast to t_bc [128, N]
    s    = xb @ w0 -> per-partition bias per i-tile     [128, NIT]
  main loop over the core's 8 row-tiles (128 rows each):
    u     = Prelu(t_bc + s_i)         (ScalarE, slope 0.2, bias per-partition)
    p     = Exp(u), accum_out -> D_i  (ScalarE; row sums for softmax denom)
    alpha = p * (1/D_i)               (VectorE tensor_scalar) -> DMA out
    pT    = PE transpose of p chunks -> PSUM -> SBUF
    numT += h_j^T-free matmuls: numT[fo,i] = sum_j h[j,fo]^T p^T[j,i] (PSUM acc)
    out   = transpose(numT) * (1/D_i) -> DMA out
"""

import os
import sys

for _p in ("/opt/trn_rl_repo",):
    if os.path.isdir(_p) and _p not in sys.path:
        sys.path.insert(0, _p)

from contextlib import ExitStack

import numpy as np

import concourse.bass as bass
import concourse.tile as tile
from concourse import mybir
from concourse.masks import make_identity

AF = mybir.ActivationFunctionType
F32 = mybir.dt.float32

N_TOT = 8192
F_IN = 256
F_OUT = 128
N_CORES = 8
BLK = N_TOT // N_CORES
SLOPE = 0.2


def build_nc(n_tot=N_TOT, blk=BLK, f_in=F_IN, f_out=F_OUT, slope=SLOPE,
             legalize=True, iters=1):
    """Build the single-core SPMD Bass program (same program on all cores;
    per-core behavior differs only through the `xb` input slice)."""
    P = 128
    assert f_out == P and f_in % P == 0
    FI_C = f_in // P            # contraction chunks over F_IN (2)
    NJT = n_tot // P            # source-node (column) tiles (64)
    NIT = blk // P              # this core's dest-row tiles (8)
    JH = n_tot // 2             # j extent per half in the main loop
    GRP = 4                     # 128-wide j-chunks per PSUM bank group
    NGH = JH // (GRP * P)       # transpose groups per half

    nc = bass.Bass()

    x_d = nc.dram_tensor("x", [n_tot, f_in], F32, kind="ExternalInput")
    xb_d = nc.dram_tensor("xb", [blk, f_in], F32, kind="ExternalInput")
    W_d = nc.dram_tensor("W", [f_out, f_in], F32, kind="ExternalInput")
    a0_d = nc.dram_tensor("a0", [f_out, 1], F32, kind="ExternalInput")
    a1_d = nc.dram_tensor("a1", [f_out, 1], F32, kind="ExternalInput")
    alpha_d = nc.dram_tensor("alpha_blk", [blk, n_tot], F32, kind="ExternalOutput")
    out_d = nc.dram_tensor("out_blk", [blk, f_out], F32, kind="ExternalOutput")

    with tile.TileContext(nc) as tc, ExitStack() as ctx:
        const = ctx.enter_context(tc.tile_pool(name="const", bufs=1))

        ident = const.tile([P, P], F32)
        make_identity(nc, ident)
        W_s = const.tile([P, f_in], F32)
        nc.sync.dma_start(out=W_s, in_=W_d[:, :])
        a0_s = const.tile([P, 1], F32)
        nc.sync.dma_start(out=a0_s, in_=a0_d[:, :])
        a1_s = const.tile([P, 1], F32)
        nc.sync.dma_start(out=a1_s, in_=a1_d[:, :])
        ones_s = const.tile([1, P], F32)
        nc.vector.memset(ones_s, 1.0)

        WT_s = const.tile([P, f_in], F32)       # W^T, chunk c at [:, c*P:(c+1)*P]
        w0_s = const.tile([P, FI_C], F32)       # W^T @ a0, chunk c at [:, c]
        w1_s = const.tile([P, FI_C], F32)
        h_s = const.tile([P, NJT * P], F32)     # h natural: [j%P, (j//P)*P + fo]
        s_s = const.tile([P, NIT], F32)         # per-row bias s_i for our block
        t_bc = const.tile([P, n_tot], F32)      # t broadcast across partitions

        # ---------------- setup ----------------
        with tc.tile_pool(name="su_ps", bufs=2, space="PSUM") as sps:
            # W^T via PE transpose
            wt_ps = sps.tile([P, f_in], F32, tag="wt")
            for c in range(FI_C):
                nc.tensor.transpose(
                    wt_ps[:, c * P:(c + 1) * P], W_s[:, c * P:(c + 1) * P], ident
                )
            nc.vector.tensor_copy(out=WT_s, in_=wt_ps)

            # w0 = W^T a0, w1 = W^T a1 (chunked over fi)
            w01_ps = sps.tile([P, 2 * FI_C], F32, tag="w01")
            for c in range(FI_C):
                nc.tensor.matmul(
                    w01_ps[:, c:c + 1], lhsT=W_s[:, c * P:(c + 1) * P], rhs=a0_s,
                    start=True, stop=True, skip_group_check=True,
                )
                nc.tensor.matmul(
                    w01_ps[:, FI_C + c:FI_C + c + 1],
                    lhsT=W_s[:, c * P:(c + 1) * P], rhs=a1_s,
                    start=True, stop=True, skip_group_check=True,
                )
            nc.vector.tensor_copy(out=w0_s, in_=w01_ps[:, 0:FI_C])
            nc.vector.tensor_copy(out=w1_s, in_=w01_ps[:, FI_C:2 * FI_C])

        with tc.tile_pool(name="xT_sb", bufs=1) as xtp:
            # xT chunks: xT[c] = x[:, c*P:(c+1)*P].T, shape [P(fi), n_tot(j)]
            xT = [xtp.tile([P, n_tot], F32, tag=f"xT{c}", name=f"xT{c}")
                  for c in range(FI_C)]

            with tc.tile_pool(name="xl_sb", bufs=3) as xlp, \
                 tc.tile_pool(name="xt_ps", bufs=2, space="PSUM") as xtps, \
                 tc.tile_pool(name="h_ps", bufs=2, space="PSUM") as hps:
                NG = NJT // GRP
                for g in range(NG):
                    pts = [xtps.tile([P, GRP * P], F32, tag=f"xt{c}", name=f"xt{c}")
                           for c in range(FI_C)]
                    for k in range(GRP):
                        jt = g * GRP + k
                        x_t = xlp.tile([P, f_in], F32, tag="x")
                        nc.sync.dma_start(out=x_t, in_=x_d[jt * P:(jt + 1) * P, :])
                        for c in range(FI_C):
                            nc.tensor.transpose(
                                pts[c][:, k * P:(k + 1) * P],
                                x_t[:, c * P:(c + 1) * P], ident,
                            )
                    # split PSUM->SBUF copies between DVE and ACT
                    nc.vector.tensor_copy(
                        out=xT[0][:, g * GRP * P:(g + 1) * GRP * P], in_=pts[0])
                    nc.scalar.copy(
                        out=xT[1][:, g * GRP * P:(g + 1) * GRP * P], in_=pts[1])

                # h natural: h[j, fo] accumulated over fi chunks
                for g in range(NG):
                    h_ps = hps.tile([P, GRP * P], F32, tag="h")
                    for k in range(GRP):
                        jt = g * GRP + k
                        for c in range(FI_C):
                            nc.tensor.matmul(
                                h_ps[:, k * P:(k + 1) * P],
                                lhsT=xT[c][:, jt * P:(jt + 1) * P],
                                rhs=WT_s[:, c * P:(c + 1) * P],
                                start=(c == 0), stop=(c == FI_C - 1),
                                skip_group_check=True,
                            )
                    nc.vector.tensor_copy(
                        out=h_s[:, g * GRP * P:(g + 1) * GRP * P], in_=h_ps)

            # t row vector [1, n_tot] then PE-broadcast into t_bc [P, n_tot]
            with tc.tile_pool(name="t_ps", bufs=2, space="PSUM") as tps, \
                 tc.tile_pool(name="t_sb", bufs=1) as tsb, \
                 tc.tile_pool(name="bc_ps", bufs=2, space="PSUM") as bcps:
                t_free = tsb.tile([1, n_tot], F32)
                NT5 = n_tot // 512
                for n in range(NT5):
                    t_ps = tps.tile([1, 512], F32, tag="t")
                    for c in range(FI_C):
                        nc.tensor.matmul(
                            t_ps, lhsT=w1_s[:, c:c + 1],
                            rhs=xT[c][:, n * 512:(n + 1) * 512],
                            start=(c == 0), stop=(c == FI_C - 1),
                            skip_group_check=True,
                        )
                    nc.scalar.copy(out=t_free[:, n * 512:(n + 1) * 512], in_=t_ps)
                for n in range(NT5):
                    bc_ps = bcps.tile([P, 512], F32, tag="bc")
                    nc.tensor.matmul(
                        bc_ps, lhsT=ones_s, rhs=t_free[:, n * 512:(n + 1) * 512],
                        start=True, stop=True, skip_group_check=True,
                    )
                    nc.vector.tensor_copy(
                        out=t_bc[:, n * 512:(n + 1) * 512], in_=bc_ps)

        # s_i for our row block: s = xb @ w0, via transposed xb chunks
        with tc.tile_pool(name="s_ps", bufs=2, space="PSUM") as ssps, \
             tc.tile_pool(name="s_sb", bufs=2) as ssb:
            xbT = [ssb.tile([P, blk], F32, tag=f"xbT{c}", name=f"xbT{c}")
                   for c in range(FI_C)]
            NGB = NIT // GRP if NIT >= GRP else 1
            BG = min(GRP, NIT)
            for g in range(NGB):
                pts = [ssps.tile([P, BG * P], F32, tag=f"bt{c}", name=f"bt{c}")
                       for c in range(FI_C)]
                for k in range(BG):
                    it = g * BG + k
                    xb_t = ssb.tile([P, f_in], F32, tag="xb")
                    nc.sync.dma_start(out=xb_t, in_=xb_d[it * P:(it + 1) * P, :])
                    for c in range(FI_C):
                        nc.tensor.transpose(
                            pts[c][:, k * P:(k + 1) * P],
                            xb_t[:, c * P:(c + 1) * P], ident,
                        )
                for c in range(FI_C):
                    nc.vector.tensor_copy(
                        out=xbT[c][:, g * BG * P:(g + 1) * BG * P], in_=pts[c])
            s_ps = ssps.tile([P, NIT], F32, tag="s")
            for it in range(NIT):
                for c in range(FI_C):
                    nc.tensor.matmul(
                        s_ps[:, it:it + 1],
                        lhsT=xbT[c][:, it * P:(it + 1) * P],
                        rhs=w0_s[:, c:c + 1],
                        start=(c == 0), stop=(c == FI_C - 1),
                        skip_group_check=True,
                    )
            nc.vector.tensor_copy(out=s_s, in_=s_ps)

        # ---------------- main loop ----------------
        upool = ctx.enter_context(tc.tile_pool(name="u", bufs=1))
        ppool = ctx.enter_context(tc.tile_pool(name="p", bufs=3))
        apool = ctx.enter_context(tc.tile_pool(name="al", bufs=2))
        dpool = ctx.enter_context(tc.tile_pool(name="d", bufs=2))
        ptsb = ctx.enter_context(tc.tile_pool(name="pts", bufs=3))
        opool = ctx.enter_context(tc.tile_pool(name="o", bufs=2))
        ptps = ctx.enter_context(tc.tile_pool(name="pt_ps", bufs=2, space="PSUM"))
        numps = ctx.enter_context(tc.tile_pool(name="num_ps", bufs=2, space="PSUM"))
        nops = ctx.enter_context(tc.tile_pool(name="no_ps", bufs=2, space="PSUM"))

        for it in [i % NIT for i in range(NIT * iters)]:
            s_bias = s_s[:, it:it + 1]
            D_t = dpool.tile([P, 2], F32, tag="D")
            numT_ps = numps.tile([P, P], F32, tag="numT")
            p_halves = []
            for hf in range(2):
                u_h = upool.tile([P, JH], F32, tag="u")
                nc.scalar.activation(
                    out=u_h, in_=t_bc[:, hf * JH:(hf + 1) * JH],
                    func=AF.Prelu, bias=s_bias, scale=1.0, alpha=slope,
                )
                p_h = ppool.tile([P, JH], F32, tag="p")
                nc.scalar.activation(
                    out=p_h, in_=u_h, func=AF.Exp,
                    accum_out=D_t[:, hf:hf + 1],
                )
                p_halves.append(p_h)
                # transpose p chunks and accumulate numT[fo, i] over j
                for g in range(NGH):
                    pt_ps = ptps.tile([P, GRP * P], F32, tag="pt")
                    for k in range(GRP):
                        jo = (g * GRP + k) * P
                        nc.tensor.transpose(
                            pt_ps[:, k * P:(k + 1) * P], p_h[:, jo:jo + P], ident)
                    pt_s = ptsb.tile([P, GRP * P], F32, tag="pts")
                    nc.vector.tensor_copy(out=pt_s, in_=pt_ps)
                    for k in range(GRP):
                        jc = hf * (JH // P) + g * GRP + k
                        nc.tensor.matmul(
                            numT_ps,
                            lhsT=h_s[:, jc * P:(jc + 1) * P],
                            rhs=pt_s[:, k * P:(k + 1) * P],
                            start=(jc == 0), stop=(jc == NJT - 1),
                            skip_group_check=True,
                        )
            # softmax denominator and normalization
            rd_t = dpool.tile([P, 1], F32, tag="rd")
            nc.vector.tensor_add(out=rd_t, in0=D_t[:, 0:1], in1=D_t[:, 1:2])
            nc.vector.reciprocal(out=rd_t, in_=rd_t)
            for hf in range(2):
                al_h = apool.tile([P, JH], F32, tag="al")
                nc.vector.tensor_scalar_mul(al_h, p_halves[hf], rd_t)
                nc.sync.dma_start(
                    out=alpha_d[it * P:(it + 1) * P, hf * JH:(hf + 1) * JH],
                    in_=al_h,
                )
            # out rows: transpose numT back and scale by 1/D
            numT_s = ptsb.tile([P, P], F32, tag="numTs")
            nc.vector.tensor_copy(out=numT_s, in_=numT_ps)
            num_ps = nops.tile([P, P], F32, tag="num")
            nc.tensor.transpose(num_ps, numT_s, ident)
            o_t = opool.tile([P, f_out], F32, tag="o")
            nc.vector.tensor_scalar_mul(o_t, num_ps, rd_t)
            nc.sync.dma_start(out=out_d[it * P:(it + 1) * P, :], in_=o_t)

    if legalize:
        _legalize_matmul_waits(nc)
    return nc


def _legalize_matmul_waits(nc):
    """This walrus build accepts only one sync-wait per instruction for
    several ISA templates (S3_LW matmul, PSEUDO_DMA_DIRECT2D, ...). Hoist
    excess waits onto NoOps on the same engine inserted right before the
    offending instruction (a wait moved earlier on the same engine is always
    safe)."""
    nop_id = 0
    for fn in nc.m.functions:
        for blk in fn.blocks:
            insts = blk.instructions
            patches = []
            for i, inst in enumerate(insts):
                si = getattr(inst, "sync_info", None)
                if si is None or len(si.on_wait) <= 1:
                    continue
                patches.append((i, inst))
            for i, inst in reversed(patches):
                si = inst.sync_info
                extra, keep = si.on_wait[:-1], si.on_wait[-1:]
                inst.sync_info = mybir.SyncInfo(on_wait=keep, on_update=si.on_update)
                for w in reversed(extra):
                    nop = mybir.InstNoOp(name=f"wsplit{nop_id}")
                    nop_id += 1
                    nop.engine = inst.engine
                    nop.sync_info = mybir.SyncInfo(on_wait=[w], on_update=[])
                    insts.insert(i, nop)


_NC_CACHE = None


def _get_nc():
    global _NC_CACHE
    if _NC_CACHE is None:
        _NC_CACHE = build_nc()
    return _NC_CACHE


def make_in_maps(x, W, a, n_cores=N_CORES, blk=BLK, f_out=F_OUT):
    x = np.ascontiguousarray(np.asarray(x, dtype=np.float32))
    W = np.ascontiguousarray(np.asarray(W, dtype=np.float32))
    a = np.ascontiguousarray(np.asarray(a, dtype=np.float32))
    a0 = a[:f_out].reshape(f_out, 1).copy()
    a1 = a[f_out:].reshape(f_out, 1).copy()
    return [
        {
            "x": x,
            "xb": np.ascontiguousarray(x[c * blk:(c + 1) * blk]),
            "W": W,
            "a0": a0,
            "a1": a1,
        }
        for c in range(n_cores)
    ]


def kernel(x, W, a):
    from concourse.bass_utils import run_bass_kernel_spmd

    nc = _get_nc()
    in_maps = make_in_maps(x, W, a)
    res = run_bass_kernel_spmd(nc, in_maps, list(range(N_CORES)))
    outs = res.results
    alpha = np.concatenate([outs[c]["alpha_blk"] for c in range(N_CORES)], axis=0)
    out = np.concatenate([outs[c]["out_blk"] for c in range(N_CORES)], axis=0)
    return out, alpha
